# revision 32
# baseline (speedup 1.0000x reference)
"""Trainium2 Bass kernel for nn_AuxLoss (aux CE loss + erf regularizer, segment-
mean over K=10 classes), data-parallel over 8 NeuronCores.

Math (per reference):
  f(u)      = 0.5 - 0.5*erf((-0.5 - u)/(sigma*sqrt2)) = 0.5 + 0.5*erf(sqrt2*u + sqrt2/2)
  row_reg_n = sum_d f(u[n,d])
  row_ce_n  = logsumexp(yg[n,:]) - yg[n, yhat[n]]
  per-class means over rows with yhat==k, averaged over present classes:
  out = aux + lmbd * reg

Final design (125 us HW, vs 195 us baseline). Measured facts: the dual-HWDGE
wire sustains ~340-425 GB/s under this issue pattern, the single ACT engine
needs ~75 us (erf 16 x 3.71 us at work-stride <=67, exp 8 x 1.63 us, ln 1.2
us), and per-engine DMA issues stall on semaphore recycling after ~3
outstanding transfers. The kernel keeps both queues backlogged and the ACT
stream wire-paced:
  - ACT chain: warm-exp, exp x8 (one per 128-row yg piece, gated only on its
    own DMA), ln, then erf c0..c15 back-to-back (first and last chunks in 2
    pieces: the first so erf starts as soon as 1 MB has landed, the last to
    shorten the PE tail). exp and ln share one ACT table set, so the whole
    kernel pays exactly 3 table loads (exp prewarmed at t~0, ln, erf).
    Because ln completes before the first erf, lse can ride the work tile
    like every other per-row quantity -- no separate segment path at all.
  - Scheduler-proofing (v5-v8 deadlocked or mis-scheduled): u DMA issues for
    chunks >= u_bufs carry a branch dependency on the erf whose completion
    frees their pool slot, so the counting cap-gate is satisfied by
    construction and slots are requested in chunk order; every DVE
    instruction sits on one explicit chain in dependency-forward order. No
    dependency ever lands ON a DMA instruction (that waits for the
    *transfer*, which cost v2/v3 ~35 us of ACT delay).
  - DMA: scalar HWDGE queue carries u0 (2 pieces) + u1 at t0 (a 4th t0
    issue would hit semaphore recycling ahead of the ACT stream), u4/u5
    branch-dep'd on early exps, then odd tail chunks; sync HWDGE queue
    carries the 8 yg pieces (full-resident pool, no WARs), u2/u3, then even
    tail chunks. Tail chunks alternate queues; u_bufs=6 keeps ~3 transfers
    outstanding per queue. yhat rides the gpsimd SWDGE queue, casting
    i32->f32 in flight.
  - NO on-device collective: each core emits its raw [10, 67] f32 segment
    accumulator; the host sums the 8 cores and does the ~50-flop finish in
    numpy (the legitimate unshard step). v1's AllGather cost a ~44 us tail.
  - per 64-row chunk one bf16 work tile [128, 64, 67]:
      cols 0:64  erf(sqrt2*u + sqrt2/2)    (ACT, strided out; the 0.5+0.5*
                 affine is folded into the host fixup)
      col  64    picked = sum_c onehot*yg  (DVE row-reduce of onehot*raw yg
                                            straight into the work column)
      col  65    ones                      (counts)
      col  66    lse                       (DVE bf16 copy from the ln output)
  - PE: per 128-row group ldweights(onehot[128,10]) + matmul(work[128,67]),
    accumulating PSUM [10,67] over all 1024 groups; onehot lives in one big
    bf16 tile computed early from yhat via iota-compare.
"""

import math
import sys

if "/opt/trn_rl_repo" not in sys.path:
    sys.path.insert(0, "/opt/trn_rl_repo")

import numpy as np

N_CORES = 8
N_FULL = 1048576
C = 10
D = 64
P = 128
ROWS_PER_CORE = N_FULL // N_CORES  # 131072
SQ2 = math.sqrt(2.0)
W_COLS = D + 3  # erf block | picked | ones | lse
W_OUT = W_COLS

COL_PICK = D
COL_ONES = D + 1
COL_LSE = D + 2


def build(rows_per_core=ROWS_PER_CORE, w=64, n_slab=4, n_half=8, u_bufs=6,
          w_bufs=3, yge_bufs=2):
    from concourse import bacc, mybir, tile

    f32 = mybir.dt.float32
    bf16 = mybir.dt.bfloat16
    i32 = mybir.dt.int32
    FT = mybir.ActivationFunctionType
    ALU = mybir.AluOpType
    AX = mybir.AxisListType

    rpp = rows_per_core // P  # rows per partition (1024)
    assert rpp * P == rows_per_core
    nch = rpp // w  # chunks (16)
    assert nch * w == rpp
    slab = rpp // n_slab  # lse slab rows (256)
    assert slab * n_slab == rpp
    half = rpp // n_half  # yg piece rows (128)
    assert half * n_half == rpp
    assert w <= half and half % w == 0

    nc = bacc.Bacc("TRN2", target_bir_lowering=False, debug=False, num_devices=N_CORES)

    yh_d = nc.dram_tensor("yhat", [rows_per_core], i32, kind="ExternalInput")
    yg_d = nc.dram_tensor("yg", [rows_per_core, C], f32, kind="ExternalInput")
    u_d = nc.dram_tensor("u_zg", [rows_per_core, D], f32, kind="ExternalInput")
    out_d = nc.dram_tensor("out", [C, W_OUT], f32, kind="ExternalOutput")

    u_v = u_d[:].rearrange("(p r) d -> p r d", p=P)
    yg_v = yg_d[:].rearrange("(p r) c -> p r c", p=P)
    yh_v = yh_d[:].rearrange("(p r) -> p r", p=P)

    from concourse.tile_rust import add_dep_helper

    def mk_chain(box, reason):
        def link(inst):
            raw = getattr(inst, "ins", inst)
            if box[0] is not None:
                add_dep_helper(raw, box[0], sync=True, reason=reason)
            box[0] = raw
            return inst
        return link

    sc_box = [None]
    sc_ordered = mk_chain(sc_box, "act order")
    dve_ordered = mk_chain([None], "dve order")
    gp_ordered = mk_chain([None], "gpsimd order")

    with tile.TileContext(nc) as tc:
        with (
            tc.tile_pool(name="const", bufs=1) as constp,
            tc.tile_pool(name="io", bufs=1) as iop,
            tc.tile_pool(name="work", bufs=1) as workp,
            tc.tile_pool(name="psum", bufs=1, space="PSUM") as psump,
        ):
            # --- u tiles created in ci order. Chunks 0..3 use fresh pool
            # slots and are issued at t0; every chunk k >= u_bufs is issued
            # with a branch dependency on erf(k - u_bufs) -- the compute
            # instruction whose completion frees its slot -- so the counting
            # cap-gate is always already satisfied when an issue runs and
            # slots are requested in ci order on every engine (v6/v7
            # deadlocked when the scheduler let a late chunk's issue grab a
            # slot ahead of an early one, or parked an issue with an
            # unsatisfied cap-wait inside the ACT stream). No dependency
            # ever lands ON a DMA instruction (that would wait for the
            # transfer, which cost v2/v3 ~35 us of ACT delay). ---
            u_ts = {ci: iop.tile([P, w, D], f32, name="u_t", bufs=u_bufs)
                    for ci in range(nch)}
            h2 = w // 2

            def is_scalar_chunk(ci):
                return ci <= 1 or ci in (4, 5) or (ci >= u_bufs and ci % 2 == 1)

            def issue_u(ci, piece=None, after=None):
                eng = nc.scalar if is_scalar_chunk(ci) else nc.sync
                if piece is None:
                    r0, r1 = 0, w
                else:
                    r0, r1 = piece * h2, (piece + 1) * h2
                inst = eng.dma_start(
                    u_ts[ci][:, r0:r1, :],
                    u_v[:, ci * w + r0 : ci * w + r1, :],
                )
                if after is not None:
                    add_dep_helper(
                        getattr(inst, "ins", inst), after, sync=True,
                        reason="u slot freed by this erf",
                    )
                return inst

            # t0: ONLY 3 issues on the scalar engine -- a 4th stalls on DMA
            # semaphore recycling and, being ahead of the ACT stream, would
            # block warm+exp. First a single DMA covering yg pieces 0..1 (so
            # the exp stream starts as early as possible), then u0, u1. The
            # sync engine may stall freely: yg pieces 2..7, then u2, u3.
            yg_a = iop.tile([P, 2 * half, C], f32, name="yg_a", bufs=1)
            nc.scalar.dma_start(yg_a[:], yg_v[:, 0 : 2 * half, :])
            issue_u(0)
            issue_u(1)
            yg_ts = {}
            for h in range(2, n_half):
                yg_t = iop.tile([P, half, C], f32, name="yg_t", bufs=n_half - 2)
                nc.sync.dma_start(yg_t[:], yg_v[:, h * half : (h + 1) * half, :])
                yg_ts[h] = yg_t
            for ci in (2, 3):
                issue_u(ci)

            def yg_piece(h):
                if h < 2:
                    return yg_a[:, h * half : (h + 1) * half, :]
                return yg_ts[h][:]
            # yhat via gpsimd SWDGE, casting i32 -> f32 in flight
            yh_f = constp.tile([P, rpp], f32)
            gp_ordered(nc.gpsimd.dma_start(yh_f[:], yh_v))

            # --- constants ---
            erf_bias = constp.tile([P, 1], f32)
            nc.vector.memset(erf_bias[:], 0.5 * SQ2)
            iota_f = constp.tile([P, 1, C], f32)
            gp_ordered(nc.gpsimd.iota(
                iota_f[:, 0, :], [[1, C]],
                channel_multiplier=0, allow_small_or_imprecise_dtypes=True,
            ))

            # warm the exp table while the first DMAs are in flight
            warm_act = constp.tile([1, 1], f32)
            nc.vector.memset(warm_act[:], 1.0)
            wa_o = constp.tile([1, 1], f32)
            sc_ordered(nc.scalar.activation(wa_o[:], warm_act[:], FT.Exp))

            # --- ACT phase 1: exp per yg piece, then one ln (shared table).
            # The DVE sumexp reduces MUST be emitted before the ln: the Tile
            # dep tracker orders accesses by emission, so a read emitted
            # before its writers gets no dependency (this was the v9/v10
            # NaN). The onehot for slab 0 is emitted first so it heads the
            # DVE chain. ---
            sume = constp.tile([P, rpp], f32)
            lse16 = constp.tile([P, rpp], f32)
            ohbig = constp.tile([P, rpp, C], bf16)

            def do_oneh(s):
                s0, s1 = s * slab, (s + 1) * slab
                dve_ordered(nc.vector.tensor_tensor(
                    ohbig[:, s0:s1, :],
                    yh_f[:, s0:s1].broadcast_to([P, slab, C]),
                    iota_f[:].broadcast_to([P, slab, C]),
                    ALU.is_equal,
                ))

            yge_ts = {}
            exp_insts = []
            for h in range(n_half):
                yge = workp.tile([P, half, C], bf16, name="yge", bufs=yge_bufs)
                ei = sc_ordered(nc.scalar.activation(yge[:], yg_piece(h), FT.Exp))
                exp_insts.append(getattr(ei, "ins", ei))
                yge_ts[h] = yge
                h0 = h * half
                dve_ordered(nc.vector.reduce_sum(
                    sume[:, h0 : h0 + half], yge[:], axis=AX.X
                ))
            issue_u(4, after=exp_insts[1])
            issue_u(5, after=exp_insts[3])
            sc_ordered(nc.scalar.activation(lse16[:], sume[:], FT.Ln))
            do_oneh(0)

            # --- ACT phase 2: the erf stream with chained scalar u issues ---
            work_ts = {}

            def do_erf(ci, parts=1):
                work_t = workp.tile([P, w, W_COLS], bf16, name="work_t", bufs=w_bufs)
                u_t = u_ts.pop(ci)
                step = w // parts
                for k in range(parts):
                    r0, r1 = k * step, (k + 1) * step
                    sc_ordered(
                        nc.scalar.activation(
                            work_t[:, r0:r1, 0:D], u_t[:, r0:r1, :], FT.Erf,
                            bias=erf_bias[:], scale=SQ2,
                        )
                    )
                work_ts[ci] = work_t

            for ci in range(nch):
                do_erf(ci, parts=4 if ci == nch - 1 else 1)
                nxt = ci + u_bufs
                if nxt < nch:
                    issue_u(nxt, after=sc_box[0])

            # --- DVE chain continues: side cols c0..c3, oneh s1..s3, side
            # cols c4..c15, accS ---
            def do_side(ci):
                r0 = ci * w
                h = ci // (half // w)
                hr0 = (ci % (half // w)) * w
                pg_t = workp.tile([P, w, C], bf16, name="pg_t", bufs=1)
                dve_ordered(nc.vector.tensor_tensor(
                    pg_t[:], ohbig[:, r0 : r0 + w, :],
                    yg_piece(h)[:, hr0 : hr0 + w, :], ALU.mult,
                ))
                with nc.allow_low_precision(reason="picked row has 1 nonzero"):
                    dve_ordered(nc.vector.reduce_sum(
                        work_ts[ci][:, :, COL_PICK], pg_t[:], axis=AX.X
                    ))
                dve_ordered(nc.vector.memset(work_ts[ci][:, :, COL_ONES], 1.0))
                r0 = ci * w
                dve_ordered(nc.vector.tensor_copy(
                    work_ts[ci][:, :, COL_LSE], lse16[:, r0 : r0 + w]
                ))

            chunks_per_slab = slab // w
            for ci in range(chunks_per_slab):
                do_side(ci)
            for s in range(1, n_slab):
                do_oneh(s)
            for ci in range(chunks_per_slab, nch):
                do_side(ci)

            # --- PE segment accumulation ---
            ps = psump.tile([C, W_COLS], f32)
            for ci in range(nch):
                r0 = ci * w
                work_t = work_ts.pop(ci)
                for g in range(w):
                    first = ci == 0 and g == 0
                    last = ci == nch - 1 and g == w - 1
                    nc.tensor.matmul(
                        ps[:], ohbig[:, r0 + g, :], work_t[:, g, :],
                        start=first, stop=last,
                    )

            # --- emit the raw accumulator; host finishes ---
            accS = constp.tile([C, W_OUT], f32)
            dve_ordered(nc.vector.tensor_copy(accS[:], ps[:]))
            nc.sync.dma_start(out_d[:], accS[:])

    nc.compile()
    return nc


_NC_CACHE = {}


def _get_nc(**kw):
    key = tuple(sorted(kw.items()))
    if key not in _NC_CACHE:
        _NC_CACHE[key] = build(**kw)
    return _NC_CACHE[key]


def make_in_maps(yhat, yg, u_zg, rows_per_core=ROWS_PER_CORE):
    yhat = np.ascontiguousarray(np.asarray(yhat).astype(np.int32))
    yg = np.ascontiguousarray(np.asarray(yg, dtype=np.float32))
    u_zg = np.ascontiguousarray(np.asarray(u_zg, dtype=np.float32))
    n = yhat.shape[0]
    assert n == rows_per_core * N_CORES
    in_maps = []
    for i in range(N_CORES):
        s = slice(i * rows_per_core, (i + 1) * rows_per_core)
        in_maps.append({"yhat": yhat[s], "yg": yg[s], "u_zg": u_zg[s]})
    return in_maps


def _finish(acc_sum, lmbd):
    """acc_sum: [C, W_OUT] f64 summed over cores. ~50 flops in numpy."""
    seg_erf = acc_sum[:, 0:D].sum(axis=1)
    seg_pick = acc_sum[:, COL_PICK]
    cnt = acc_sum[:, COL_ONES]
    seg_lse = acc_sum[:, COL_LSE]
    present = cnt > 0
    denom = np.where(present, cnt, 1.0)
    seg_reg = 0.5 * D * cnt + 0.5 * seg_erf
    reg_c = seg_reg / (denom * D)
    aux_c = (seg_lse - seg_pick) / denom
    n_unique = present.sum()
    reg = np.where(present, reg_c, 0.0).sum() / n_unique
    aux = np.where(present, aux_c, 0.0).sum() / n_unique
    return np.float32(aux + float(lmbd) * reg)


def run(yhat, yg, u_zg, lmbd, trace=False, rows_per_core=ROWS_PER_CORE, **kw):
    from concourse import bass_utils

    nc = _get_nc(rows_per_core=rows_per_core, **kw)
    in_maps = make_in_maps(yhat, yg, u_zg, rows_per_core)
    res = bass_utils.run_bass_kernel_spmd(
        nc, in_maps, core_ids=list(range(N_CORES)), trace=trace
    )
    acc = np.zeros((C, W_OUT), dtype=np.float64)
    for r in res.results:
        acc += np.asarray(r["out"], dtype=np.float64)
    val = _finish(acc, lmbd)
    return val, res


def kernel(yhat, yg, u_zg, lmbd):
    val, _ = run(yhat, yg, u_zg, lmbd)
    return np.asarray(val, dtype=np.float32).reshape(())
